# revision 2
# baseline (speedup 1.0000x reference)
"""Trainium2 Bass kernel for CustomMinkowskiLayerNorm.

Math (matches the jax reference):
    counts[b]  = #points with batch_indices == b           (clamped >= 1)
    mean[b,c]  = sum_{i in b} x[i,c] / counts[b]
    var[b,c]   = sum_{i in b} (x[i,c]-mean)^2 / counts[b]  (= E[x^2]-mean^2)
    out[i,c]   = (x[i,c]-mean[b_i,c]) / sqrt(var[b_i,c]+eps) * gamma[c] + beta[c]

Sharding: batch_indices is sorted and BATCH == n_cores == 8, so each core owns
exactly one batch segment -> all segment reductions are core-local, no
collectives. The host splits at segment boundaries (searchsorted), transposes
each segment to channel-major layout and zero-pads to a fixed shape:

    xt[p, f], p in [0,128): partition p < 64  = channel p,  points [0, F_HALF)
                            partition p >= 64 = channel p-64, points [F_HALF, 2*F_HALF)

Device program (per core, identical SPMD). The kernel is DMA-bound:
~32.5 MB in + ~32.5 MB out per core at the ~430 GB/s SBUF-fabric rate is
~151 us of pure transfer. Everything else hides behind it:

  pass 1: stream [128, 2048] f32 tiles on the sync HWDGE ring. Each tile is
          immediately compressed to a RESIDENT fp16 copy (31 tiles = 124
          KB/partition, fits in SBUF) so no tile is ever re-read from HBM.
          Per tile: ScalarE activation(Copy) writes the fp16 copy and its
          accum_out yields the per-partition sum; DVE scalar_tensor_tensor
          computes x*x into a scratch with accum_out yielding the sumsq.
          Both engines run under the ~2.4 us/tile DMA delivery rate.
  stats:  reduce the per-tile (sum, sumsq) pairs; fold partitions p/p+64 and
          broadcast with one TensorE matmul against a 0/1 fold matrix; apply
          1/count; rstd = 1/sqrt(var+eps) (ACT Sqrt table + DVE reciprocal;
          fp16 storage already bounds accuracy, no Newton refinement);
          s = gamma*rstd, t = beta - mean*s.
  pass 2: out_f32 = fp16_tile * s + t (DVE tensor_scalar, per-partition
          scalars) into a rotating f32 slot, stored on the scalar HWDGE ring.

The small inputs (invn/gamma/beta/fold matrix) load on the scalar ring, which
is idle during pass 1, so the sync ring starts the big tile burst at t=0.
"""

import os
import sys

for _p in ("/opt/trn_rl_repo", "/root/.axon_site/_ro/trn_rl_repo"):
    if os.path.isdir(_p) and _p not in sys.path:
        sys.path.append(_p)

from contextlib import ExitStack

import numpy as np

import concourse.bacc as bacc
import concourse.tile as tile
from concourse import mybir
from concourse._compat import with_exitstack
from concourse.bass_utils import run_bass_kernel_spmd

F32 = mybir.dt.float32
F16 = mybir.dt.float16

N = 1_000_000
C = 64
BATCH = 8
EPS = 1e-5

P = 128            # SBUF partitions
F_TILE = 2048      # free elems per tile -> [128, 2048] f32 = 1 MiB per DMA
LOAD_BUFS = 4      # rotating pass-1 load slots (f32)
OUT_BUFS = 4       # rotating pass-2 output slots (f32)
SQ_BUFS = 2        # rotating x^2 scratch slots (fp16)

_mult = mybir.AluOpType.mult
_add = mybir.AluOpType.add

_AF = mybir.ActivationFunctionType


def _make_body(f_half: int):
    nt = f_half // F_TILE

    @with_exitstack
    def _body(ctx: ExitStack, tc: tile.TileContext,
              out_ap, xt_ap, invn_ap, gcol_ap, bcol_ap, foldm_ap):
        nc = tc.nc

        cache = ctx.enter_context(tc.tile_pool(name="cache", bufs=nt))
        lpool = ctx.enter_context(tc.tile_pool(name="lpool", bufs=LOAD_BUFS))
        opool = ctx.enter_context(tc.tile_pool(name="opool", bufs=OUT_BUFS))
        sqpool = ctx.enter_context(tc.tile_pool(name="sqpool", bufs=SQ_BUFS))
        small = ctx.enter_context(tc.tile_pool(name="small", bufs=1))
        psum = ctx.enter_context(tc.tile_pool(name="psum", bufs=1, space="PSUM"))

        accs = small.tile([P, nt, 2], F32, tag="accs")

        # Small inputs on the scalar ring: it is idle until pass 2, so these
        # do not delay the pass-1 tile burst on the sync ring.
        invn_sb = small.tile([P, 1], F32, tag="invn")
        gcol_sb = small.tile([P, 1], F32, tag="gcol")
        bcol_sb = small.tile([P, 1], F32, tag="bcol")
        foldm_sb = small.tile([P, P], F32, tag="foldm")
        nc.scalar.dma_start(out=invn_sb, in_=invn_ap)
        nc.scalar.dma_start(out=gcol_sb, in_=gcol_ap)
        nc.scalar.dma_start(out=bcol_sb, in_=bcol_ap)
        nc.scalar.dma_start(out=foldm_sb, in_=foldm_ap)

        # Pre-load the ACT sqrt table so the stats chain doesn't stall on
        # ACT_TABLE_LOAD.
        warm = small.tile([P, 1], F32, tag="warm")
        nc.vector.memset(warm, 1.0)
        nc.scalar.activation(out=warm, in_=warm, func=_AF.Sqrt)

        # ---- pass 1: stream tiles; fp16 cache + (sum, sumsq) per tile ----
        cached = []
        for t in range(nt):
            sl = slice(t * F_TILE, (t + 1) * F_TILE)
            lslot = lpool.tile([P, F_TILE], F32, tag="l")
            nc.sync.dma_start(out=lslot, in_=xt_ap[:, sl])
            c16 = cache.tile([P, F_TILE], F16, tag="c")
            cached.append(c16)
            nc.scalar.activation(out=c16, in_=lslot, func=_AF.Copy,
                                 accum_out=accs[:, t, 0:1])
            sq16 = sqpool.tile([P, F_TILE], F16, tag="sq")
            nc.vector.scalar_tensor_tensor(out=sq16, in0=lslot, scalar=1.0,
                                           in1=lslot, op0=_mult, op1=_mult,
                                           accum_out=accs[:, t, 1:2])

        # ---- aggregate: sums[p, 0] = sum x, sums[p, 1] = sum x^2 ----
        sums = small.tile([P, 2], F32, tag="sums")
        acc_view = accs.rearrange("p t c -> p c t")
        nc.vector.reduce_sum(out=sums, in_=acc_view, axis=mybir.AxisListType.X)

        # ---- fold halves + broadcast: tot[p] = sums[p%64] + sums[p%64+64] ----
        ptot = psum.tile([P, 2], F32, tag="pt")
        nc.tensor.matmul(out=ptot, lhsT=foldm_sb, rhs=sums,
                         start=True, stop=True)
        tot = small.tile([P, 2], F32, tag="tot")
        nc.vector.tensor_copy(out=tot, in_=ptot)

        # ---- per-channel coefficients ----
        mm = small.tile([P, 2], F32, tag="mm")      # (mean, E[x^2])
        nc.vector.tensor_scalar_mul(out=mm, in0=tot, scalar1=invn_sb[:, 0:1])
        var = small.tile([P, 1], F32, tag="var")
        nc.vector.tensor_mul(out=var, in0=mm[:, 0:1], in1=mm[:, 0:1])
        nc.vector.tensor_sub(out=var, in0=mm[:, 1:2], in1=var)
        v = small.tile([P, 1], F32, tag="v")
        nc.vector.tensor_scalar(out=v, in0=var, scalar1=0.0, scalar2=EPS,
                                op0=mybir.AluOpType.max, op1=_add)
        r = small.tile([P, 1], F32, tag="r")
        nc.scalar.activation(out=r, in_=v, func=_AF.Sqrt)
        nc.vector.reciprocal(out=r, in_=r)
        s_col = small.tile([P, 1], F32, tag="s_col")
        nc.vector.tensor_mul(out=s_col, in0=r, in1=gcol_sb)
        t_col = small.tile([P, 1], F32, tag="t_col")
        nc.vector.tensor_mul(out=t_col, in0=mm[:, 0:1], in1=s_col)
        nc.vector.tensor_sub(out=t_col, in0=bcol_sb, in1=t_col)

        # ---- pass 2: out = fp16_tile * s + t, store on scalar ring ----
        for t in range(nt):
            sl = slice(t * F_TILE, (t + 1) * F_TILE)
            oslot = opool.tile([P, F_TILE], F32, tag="o")
            nc.vector.tensor_scalar(out=oslot, in0=cached[t],
                                    scalar1=s_col[:, 0:1], scalar2=t_col[:, 0:1],
                                    op0=_mult, op1=_add)
            nc.scalar.dma_start(out=out_ap[:, sl], in_=oslot)

    return _body


_NC_CACHE = {}


def _build_program(f_half: int):
    if f_half in _NC_CACHE:
        return _NC_CACHE[f_half]
    nc = bacc.Bacc("TRN2", target_bir_lowering=False, debug=False,
                   num_devices=BATCH)
    xt = nc.dram_tensor("xt", [P, f_half], F32, kind="ExternalInput").ap()
    invn = nc.dram_tensor("invn", [P, 1], F32, kind="ExternalInput").ap()
    gcol = nc.dram_tensor("gcol", [P, 1], F32, kind="ExternalInput").ap()
    bcol = nc.dram_tensor("bcol", [P, 1], F32, kind="ExternalInput").ap()
    foldm = nc.dram_tensor("foldm", [P, P], F32, kind="ExternalInput").ap()
    out = nc.dram_tensor("out", [P, f_half], F32, kind="ExternalOutput").ap()
    with tile.TileContext(nc) as tc:
        _make_body(f_half)(tc, out, xt, invn, gcol, bcol, foldm)
    nc.compile()
    _NC_CACHE[f_half] = nc
    return nc


def _prepare(features, batch_indices, gamma, beta):
    features = np.asarray(features, dtype=np.float32)
    batch_indices = np.asarray(batch_indices, dtype=np.int32)
    gamma = np.asarray(gamma, dtype=np.float32)
    beta = np.asarray(beta, dtype=np.float32)

    bounds = np.searchsorted(batch_indices, np.arange(BATCH + 1), side="left")
    cnts = np.diff(bounds)
    # fixed SPMD shape: half-row length, padded to a multiple of F_TILE
    f_half = max(int(-(-int(cnts.max()) // 2 // F_TILE) * F_TILE), F_TILE)

    gcol = np.concatenate([gamma, gamma]).reshape(P, 1).astype(np.float32)
    bcol = np.concatenate([beta, beta]).reshape(P, 1).astype(np.float32)
    k = np.arange(P)
    foldm = (k[:, None] % C == k[None, :] % C).astype(np.float32)

    in_maps = []
    for b in range(BATCH):
        s, e = int(bounds[b]), int(bounds[b + 1])
        cnt = e - s
        xt = np.zeros((P, f_half), dtype=np.float32)
        n1 = min(cnt, f_half)
        if n1 > 0:
            xt[0:C, :n1] = features[s : s + n1].T
        if cnt > f_half:
            xt[C:P, : cnt - f_half] = features[s + f_half : e].T
        in_maps.append({
            "xt": xt,
            "invn": np.full((P, 1), 1.0 / max(cnt, 1), dtype=np.float32),
            "gcol": gcol,
            "bcol": bcol,
            "foldm": foldm,
        })
    return in_maps, bounds, f_half


def _assemble(results, bounds, f_half):
    out = np.empty((N, C), dtype=np.float32)
    for b in range(BATCH):
        s, e = int(bounds[b]), int(bounds[b + 1])
        cnt = e - s
        if cnt == 0:
            continue
        ot = results[b]["out"]
        n1 = min(cnt, f_half)
        out[s : s + n1] = ot[0:C, :n1].T
        if cnt > f_half:
            out[s + f_half : e] = ot[C:P, : cnt - f_half].T
    return out


def run_with_results(features, batch_indices, gamma, beta, **run_kwargs):
    in_maps, bounds, f_half = _prepare(features, batch_indices, gamma, beta)
    nc = _build_program(f_half)
    res = run_bass_kernel_spmd(nc, in_maps, core_ids=list(range(BATCH)),
                               **run_kwargs)
    return _assemble(res.results, bounds, f_half), res


def kernel(features, batch_indices, gamma, beta):
    out, _ = run_with_results(features, batch_indices, gamma, beta)
    return out


# revision 4
# speedup vs baseline: 1.2109x; 1.2109x over previous
"""Trainium2 Bass kernel for CustomMinkowskiLayerNorm.

Math (matches the jax reference):
    counts[b]  = #points with batch_indices == b           (clamped >= 1)
    mean[b,c]  = sum_{i in b} x[i,c] / counts[b]
    var[b,c]   = sum_{i in b} (x[i,c]-mean)^2 / counts[b]  (= E[x^2]-mean^2)
    out[i,c]   = (x[i,c]-mean[b_i,c]) / sqrt(var[b_i,c]+eps) * gamma[c] + beta[c]

Sharding: batch_indices is sorted and BATCH == n_cores == 8, so each core owns
exactly one batch segment -> all segment reductions are core-local, no
collectives. The host splits at segment boundaries (searchsorted), transposes
each segment to channel-major layout and zero-pads to a fixed shape:

    xt[p, f], p in [0,128): partition p < 64  = channel p,  points [0, F_HALF)
                            partition p >= 64 = channel p-64, points [F_HALF, 2*F_HALF)

Device program (per core, identical SPMD). The kernel is DMA-bound:
~32.5 MB in + ~32.5 MB out per core at the ~430 GB/s SBUF-fabric rate is
~151 us of pure transfer. Everything else hides behind it:

  pass 1: stream [128, 2048] f32 tiles on the sync HWDGE ring. Each tile is
          immediately compressed to a RESIDENT fp16 copy (31 tiles = 124
          KB/partition, fits in SBUF) so no tile is ever re-read from HBM.
          Per tile: DVE tensor_scalar (2x_2p perf mode, ~1.3us) writes the
          fp16 copy and its accum_out yields the per-partition sum; ScalarE
          activation(Square) into a PSUM scratch (~2.2us) yields the sumsq
          via accum_out. Both engines run under the ~2.4 us/tile DMA rate.
  stats:  per-tile (sum, sumsq) pairs for tiles 0..nt-2 are reduced and
          matmul-folded (partitions p/p+64 summed + broadcast via a 0/1 fold
          matrix) while the last tile is still in flight; the last tile's
          pair is folded with a second accumulating matmul, so the
          post-last-tile critical path is one matmul + a short DVE/ACT
          chain: mean/E[x^2] (reads PSUM directly), var, sqrt, reciprocal,
          then s = gamma*rstd [, t = beta - mean*s]. With the common
          gamma==1/beta==0 inputs (host-detected) the s/t ops collapse and
          pass 2 computes (x - mean) * rstd in a single tensor_scalar.
  pass 2: out_f32 = (fp16_tile - mean) * s (DVE tensor_scalar, per-partition
          scalars) into a rotating f32 slot, stored on the scalar HWDGE ring.

The small inputs (invn/gamma/beta/fold matrix) load on the scalar ring, which
is idle during pass 1, so the sync ring starts the big tile burst at t=0.
"""

import os
import sys

for _p in ("/opt/trn_rl_repo", "/root/.axon_site/_ro/trn_rl_repo"):
    if os.path.isdir(_p) and _p not in sys.path:
        sys.path.append(_p)

from contextlib import ExitStack

import numpy as np

import concourse.bacc as bacc
import concourse.tile as tile
from concourse import mybir
from concourse._compat import with_exitstack
from concourse.bass_utils import run_bass_kernel_spmd

F32 = mybir.dt.float32
F16 = mybir.dt.float16

N = 1_000_000
C = 64
BATCH = 8
EPS = 1e-5

P = 128            # SBUF partitions
F_TILE = 2048      # free elems per tile -> [128, 2048] f32 = 1 MiB per DMA
LOAD_BUFS = 4      # rotating pass-1 load slots (f32)
OUT_BUFS = 6       # rotating pass-2 output slots (f32)

_mult = mybir.AluOpType.mult
_add = mybir.AluOpType.add
_sub = mybir.AluOpType.subtract

_AF = mybir.ActivationFunctionType


def _make_body(f_half: int, simple_affine: bool):
    nt = f_half // F_TILE

    @with_exitstack
    def _body(ctx: ExitStack, tc: tile.TileContext,
              out_ap, xt_ap, invn_ap, gcol_ap, bcol_ap, foldm_ap):
        nc = tc.nc

        cache = ctx.enter_context(tc.tile_pool(name="cache", bufs=nt))
        lpool = ctx.enter_context(tc.tile_pool(name="lpool", bufs=LOAD_BUFS))
        opool = ctx.enter_context(tc.tile_pool(name="opool", bufs=OUT_BUFS))
        small = ctx.enter_context(tc.tile_pool(name="small", bufs=1))
        psum = ctx.enter_context(tc.tile_pool(name="psum", bufs=1, space="PSUM"))

        # accs[p, c, t]: c=0 sum, c=1 sumsq, per tile t
        accs = small.tile([P, 2, nt], F32, tag="accs")

        # Small inputs on the scalar ring: it is idle until pass 2, so these
        # do not delay the pass-1 tile burst on the sync ring.
        invn_sb = small.tile([P, 1], F32, tag="invn")
        gcol_sb = small.tile([P, 1], F32, tag="gcol")
        bcol_sb = small.tile([P, 1], F32, tag="bcol")
        foldm_sb = small.tile([P, P], F32, tag="foldm")
        nc.scalar.dma_start(out=invn_sb, in_=invn_ap)
        nc.scalar.dma_start(out=gcol_sb, in_=gcol_ap)
        nc.scalar.dma_start(out=bcol_sb, in_=bcol_ap)
        nc.scalar.dma_start(out=foldm_sb, in_=foldm_ap)

        # Pre-load the ACT function tables (Square for pass 1, Sqrt for the
        # stats chain) so nothing stalls on ACT_TABLE_LOAD mid-stream.
        warm = small.tile([P, 1], F32, tag="warm")
        nc.vector.memset(warm, 1.0)
        nc.scalar.activation(out=warm, in_=warm, func=_AF.Square)
        nc.scalar.activation(out=warm, in_=warm, func=_AF.Sqrt)

        sq_ps = psum.tile([P, F_TILE], F32, tag="sq")
        ptot = psum.tile([P, 2], F32, tag="pt")

        # ---- pass 1: stream tiles; fp16 cache + (sum, sumsq) per tile ----
        cached = []
        for t in range(nt):
            sl = slice(t * F_TILE, (t + 1) * F_TILE)
            lslot = lpool.tile([P, F_TILE], F32, tag="l")
            nc.sync.dma_start(out=lslot, in_=xt_ap[:, sl])
            c16 = cache.tile([P, F_TILE], F16, tag="c")
            cached.append(c16)
            nc.vector.tensor_scalar(out=c16, in0=lslot, scalar1=1.0,
                                    scalar2=0.0, op0=_mult, op1=_add,
                                    accum_out=accs[:, 0, t : t + 1])
            nc.scalar.activation(out=sq_ps, in_=lslot, func=_AF.Square,
                                 accum_out=accs[:, 1, t : t + 1])
            if t == nt - 2:
                # All-but-last partial: reduce + fold while the last tile's
                # DMA is still in flight.
                sumsA = small.tile([P, 2], F32, tag="sumsA")
                nc.vector.reduce_sum(out=sumsA, in_=accs[:, :, : nt - 1],
                                     axis=mybir.AxisListType.X)
                nc.tensor.matmul(out=ptot, lhsT=foldm_sb, rhs=sumsA,
                                 start=True, stop=False)

        # ---- fold the last tile's pair; ptot[p] now holds per-channel
        #      (sum, sumsq) totals broadcast to both partition halves ----
        nc.tensor.matmul(out=ptot, lhsT=foldm_sb, rhs=accs[:, :, nt - 1],
                         start=False, stop=True)

        # ---- per-channel coefficients ----
        mm = small.tile([P, 2], F32, tag="mm")      # (mean, E[x^2])
        nc.vector.tensor_scalar_mul(out=mm, in0=ptot, scalar1=invn_sb[:, 0:1])
        var = small.tile([P, 1], F32, tag="var")
        nc.vector.tensor_mul(out=var, in0=mm[:, 0:1], in1=mm[:, 0:1])
        nc.vector.tensor_sub(out=var, in0=mm[:, 1:2], in1=var)
        v = small.tile([P, 1], F32, tag="v")
        nc.vector.tensor_scalar(out=v, in0=var, scalar1=0.0, scalar2=EPS,
                                op0=mybir.AluOpType.max, op1=_add)
        r = small.tile([P, 1], F32, tag="r")
        nc.scalar.activation(out=r, in_=v, func=_AF.Sqrt)
        nc.vector.reciprocal(out=r, in_=r)

        if simple_affine:
            # gamma == 1, beta == 0: out = (x - mean) * rstd
            sc1, sc2 = mm[:, 0:1], r[:, 0:1]
            op0, op1 = _sub, _mult
        else:
            # out = x*s + t with s = gamma*rstd, t = beta - mean*s
            s_col = small.tile([P, 1], F32, tag="s_col")
            nc.vector.tensor_mul(out=s_col, in0=r, in1=gcol_sb)
            t_col = small.tile([P, 1], F32, tag="t_col")
            nc.vector.tensor_mul(out=t_col, in0=mm[:, 0:1], in1=s_col)
            nc.vector.tensor_sub(out=t_col, in0=bcol_sb, in1=t_col)
            sc1, sc2 = s_col[:, 0:1], t_col[:, 0:1]
            op0, op1 = _mult, _add

        # ---- pass 2: affine per tile, store on scalar ring ----
        for t in range(nt):
            sl = slice(t * F_TILE, (t + 1) * F_TILE)
            oslot = opool.tile([P, F_TILE], F32, tag="o")
            nc.vector.tensor_scalar(out=oslot, in0=cached[t],
                                    scalar1=sc1, scalar2=sc2,
                                    op0=op0, op1=op1)
            nc.scalar.dma_start(out=out_ap[:, sl], in_=oslot)

    return _body


_NC_CACHE = {}


def _build_program(f_half: int, simple_affine: bool):
    key = (f_half, simple_affine)
    if key in _NC_CACHE:
        return _NC_CACHE[key]
    nc = bacc.Bacc("TRN2", target_bir_lowering=False, debug=False,
                   num_devices=BATCH)
    xt = nc.dram_tensor("xt", [P, f_half], F32, kind="ExternalInput").ap()
    invn = nc.dram_tensor("invn", [P, 1], F32, kind="ExternalInput").ap()
    gcol = nc.dram_tensor("gcol", [P, 1], F32, kind="ExternalInput").ap()
    bcol = nc.dram_tensor("bcol", [P, 1], F32, kind="ExternalInput").ap()
    foldm = nc.dram_tensor("foldm", [P, P], F32, kind="ExternalInput").ap()
    out = nc.dram_tensor("out", [P, f_half], F32, kind="ExternalOutput").ap()
    with tile.TileContext(nc) as tc:
        _make_body(f_half, simple_affine)(tc, out, xt, invn, gcol, bcol, foldm)
    nc.compile()
    _NC_CACHE[key] = nc
    return nc


def _prepare(features, batch_indices, gamma, beta):
    features = np.asarray(features, dtype=np.float32)
    batch_indices = np.asarray(batch_indices, dtype=np.int32)
    gamma = np.asarray(gamma, dtype=np.float32)
    beta = np.asarray(beta, dtype=np.float32)

    bounds = np.searchsorted(batch_indices, np.arange(BATCH + 1), side="left")
    cnts = np.diff(bounds)
    # fixed SPMD shape: half-row length, padded to a multiple of F_TILE
    f_half = max(int(-(-int(cnts.max()) // 2 // F_TILE) * F_TILE), F_TILE)

    simple_affine = bool(np.all(gamma == 1.0) and np.all(beta == 0.0))

    gcol = np.concatenate([gamma, gamma]).reshape(P, 1).astype(np.float32)
    bcol = np.concatenate([beta, beta]).reshape(P, 1).astype(np.float32)
    k = np.arange(P)
    foldm = (k[:, None] % C == k[None, :] % C).astype(np.float32)

    in_maps = []
    for b in range(BATCH):
        s, e = int(bounds[b]), int(bounds[b + 1])
        cnt = e - s
        xt = np.zeros((P, f_half), dtype=np.float32)
        n1 = min(cnt, f_half)
        if n1 > 0:
            xt[0:C, :n1] = features[s : s + n1].T
        if cnt > f_half:
            xt[C:P, : cnt - f_half] = features[s + f_half : e].T
        in_maps.append({
            "xt": xt,
            "invn": np.full((P, 1), 1.0 / max(cnt, 1), dtype=np.float32),
            "gcol": gcol,
            "bcol": bcol,
            "foldm": foldm,
        })
    return in_maps, bounds, f_half, simple_affine


def _assemble(results, bounds, f_half):
    out = np.empty((N, C), dtype=np.float32)
    for b in range(BATCH):
        s, e = int(bounds[b]), int(bounds[b + 1])
        cnt = e - s
        if cnt == 0:
            continue
        ot = results[b]["out"]
        n1 = min(cnt, f_half)
        out[s : s + n1] = ot[0:C, :n1].T
        if cnt > f_half:
            out[s + f_half : e] = ot[C:P, : cnt - f_half].T
    return out


def run_with_results(features, batch_indices, gamma, beta, **run_kwargs):
    in_maps, bounds, f_half, simple_affine = _prepare(
        features, batch_indices, gamma, beta)
    nc = _build_program(f_half, simple_affine)
    res = run_bass_kernel_spmd(nc, in_maps, core_ids=list(range(BATCH)),
                               **run_kwargs)
    return _assemble(res.results, bounds, f_half), res


def kernel(features, batch_indices, gamma, beta):
    out, _ = run_with_results(features, batch_indices, gamma, beta)
    return out
